# revision 31
# baseline (speedup 1.0000x reference)
"""Multi-head attention (B=1, S=4096, d_model=768, 12 heads) on 8 trn2 NeuronCores.

Sharding: tensor-parallel over heads (4 groups x 3 heads) x sequence-parallel over
queries (2 halves of 2048).  Core c = (g, s) with g = c // 2, s = c % 2 computes
heads 3g..3g+2 for queries s*2048..(s+1)*2048 against all 4096 keys.

Per-core device pipeline (all matmuls bf16, fp32 PSUM accumulation):
  1. Project Q^T, K^T, V from host-transposed bf16 activations; Q/K head pairs
     col-packed on disjoint PE quadrants (concurrent sub-array matmuls must
     write disjoint PSUM partitions - overlapping-output row-packing faults).
  2. scores = Q K^T per (head, 128-query tile), diagonally packed (quadrants
     (0,0)+(64,64)); exp on ScalarE with fused 1/8 scale and fused row-sum
     accumulation (softmax denominators); max-subtraction skipped (scores
     ~ N(0,1), exp is range-safe in fp32).
  3. Normalize with DVE tensor_scalar (4x bf16 mode) -> attn to HBM as bf16.
  4. Transpose attn for PV: 1/4 via DMA x-bar transpose, 3/4 via regular
     matmuls against identity (transpose_mode does not count as PE activity
     for the HAM clock gate and chills the clock to 1.2 GHz; real matmuls
     keep it at 2.4).  PV even/odd-ktile column packing -> out^T per head.
  5. w_o projection (K=64 chains), emission deferred past the next query
     chunk's first head so the PE queue never drains at chunk boundaries.

Host: slices/transposes/bf16-casts inputs (layout only), gathers attn shards
(bf16->fp32), sums the 4 head-group partials of out (the w_o "all-reduce").
"""

import numpy as np

import concourse.bass as bass
import concourse.tile as tile
from concourse import mybir
from concourse.bass_utils import run_bass_kernel_spmd
from concourse.masks import make_identity

FP32 = mybir.dt.float32
BF16 = mybir.dt.bfloat16

DM = 768          # d_model
NH = 12           # total heads
DK = 64           # head dim
HP = 3            # heads per core
HD = HP * DK      # 192: head dims per core
S = 4096          # keys (full sequence)
Q = 2048          # queries per core
P = 128
NQT = Q // P      # 16 query tiles
NKT = S // P      # 32 key tiles
DMT = DM // P     # 6 contraction tiles


def _split_multiwaits(nc):
    """The walrus in this container encodes at most one sync wait per
    instruction; split extra waits onto same-engine NoOps placed before."""
    for blk in nc.m.functions[0].blocks:
        new = []
        for inst in blk.instructions:
            si = inst.sync_info
            if si is not None and si.on_wait and len(si.on_wait) > 1:
                waits = list(si.on_wait)
                for extra in waits[:-1]:
                    nop = mybir.InstNoOp(name=f"WS-{nc.next_id()}", ins=[], outs=[])
                    nop.engine = inst.engine
                    nop.sync_info = mybir.SyncInfo(on_wait=[extra], on_update=[])
                    new.append(nop)
                inst.sync_info = mybir.SyncInfo(
                    on_wait=[waits[-1]], on_update=list(si.on_update))
            new.append(inst)
        blk.instructions = new


def build_kernel(split=True):
    nc = bass.Bass(trn_type="TRN2")

    xqT = nc.dram_tensor("xqT", [DMT, P, Q], BF16, kind="ExternalInput")
    xkT = nc.dram_tensor("xkT", [DMT, P, S], BF16, kind="ExternalInput")
    xvT = nc.dram_tensor("xvT", [DMT, P, S], BF16, kind="ExternalInput")
    wqT = nc.dram_tensor("wqT", [DMT, P, HD], BF16, kind="ExternalInput")
    wkT = nc.dram_tensor("wkT", [DMT, P, HD], BF16, kind="ExternalInput")
    wvT = nc.dram_tensor("wvT", [DMT, P, HD], BF16, kind="ExternalInput")
    woT = nc.dram_tensor("woT", [HD, DM], BF16, kind="ExternalInput")

    attn_out = nc.dram_tensor("attn_out", [HP * Q, S], BF16, kind="ExternalOutput")
    out_part = nc.dram_tensor("out_part", [Q, DM], FP32, kind="ExternalOutput")

    with tile.TileContext(nc) as tc:
        from contextlib import ExitStack

        with ExitStack() as ctx:
            kernel_body(ctx, tc, nc, xqT, xkT, xvT, wqT, wkT, wvT, woT,
                        attn_out, out_part)
    if split:
        _split_multiwaits(nc)
    return nc


def kernel_body(ctx, tc, nc, xqT, xkT, xvT, wqT, wkT, wvT, woT,
                attn_out, out_part):
    Exp = mybir.ActivationFunctionType.Exp

    wpool = ctx.enter_context(tc.tile_pool(name="wpool", bufs=1))
    xbf = ctx.enter_context(tc.tile_pool(name="xbf", bufs=3))
    qkv = ctx.enter_context(tc.tile_pool(name="qkv", bufs=1))
    unorm_p = ctx.enter_context(tc.tile_pool(name="unorm", bufs=2))
    anorm_p = ctx.enter_context(tc.tile_pool(name="anorm", bufs=3))
    anT_p = ctx.enter_context(tc.tile_pool(name="anT", bufs=2))
    outT_p = ctx.enter_context(tc.tile_pool(name="outT", bufs=2))
    omrg_p = ctx.enter_context(tc.tile_pool(name="omrg", bufs=4))
    ofin_p = ctx.enter_context(tc.tile_pool(name="ofin", bufs=2))
    dpool = ctx.enter_context(tc.tile_pool(name="dpool", bufs=8))

    spsum = ctx.enter_context(tc.tile_pool(name="spsum", bufs=2, space="PSUM"))
    tpsum = ctx.enter_context(tc.tile_pool(name="tpsum", bufs=2, space="PSUM"))
    vpsum = ctx.enter_context(tc.tile_pool(name="vpsum", bufs=2, space="PSUM"))

    const = ctx.enter_context(tc.tile_pool(name="const", bufs=1))
    ident = const.tile([P, P], BF16)
    make_identity(nc, ident)

    # --- load weights (host pre-cast to bf16) ---
    w_bf = {}
    for name, wsrc in (("q", wqT), ("k", wkT), ("v", wvT)):
        wb = wpool.tile([P, DMT, HD], BF16, tag=f"w{name}")
        nc.sync.dma_start(wb[:], wsrc.rearrange("t p c -> p t c"))
        w_bf[name] = wb

    wo_bf = []
    for h in range(HP):
        wob = wpool.tile([DK, DM], BF16, tag=f"wo{h}")
        nc.sync.dma_start(wob[:], woT[h * DK:(h + 1) * DK, :])
        wo_bf.append(wob)

    # --- resident activations ---
    QT = qkv.tile([P, HP * Q], BF16, tag="QT")    # [64 lo | 64 hi dup], per head
    KT = qkv.tile([P, HP * S], BF16, tag="KT")
    V = qkv.tile([P, NKT * HD], BF16, tag="V")    # per ktile: [128 k, 192 hd]

    # --- projections: stream x^T chunks of 512 seq positions ---
    def proj_chunks(src, n_seq, out_qk, do_v):
        n_ch = n_seq // 512
        for c in range(n_ch):
            xb = xbf.tile([P, DMT, 512], BF16, tag="xbf")
            nc.sync.dma_start(
                xb[:], src[:, :, c * 512:(c + 1) * 512].rearrange("t p c -> p t c"))
            if out_qk is not None:
                wb = w_bf["q" if n_seq == Q else "k"]
                # heads 0+1 col-packed (disjoint output partitions), head 2 solo
                ps = vpsum.tile([P, 512], FP32, tag="pv")
                for it in range(DMT):
                    nc.tensor.matmul(
                        ps[0:DK, :], lhsT=wb[:, it, 0:DK], rhs=xb[:, it, :],
                        start=(it == 0), stop=(it == DMT - 1),
                        tile_position=(0, 0), skip_group_check=True)
                    nc.tensor.matmul(
                        ps[DK:P, :], lhsT=wb[:, it, DK:2 * DK], rhs=xb[:, it, :],
                        start=(it == 0), stop=(it == DMT - 1),
                        tile_position=(0, DK), skip_group_check=True)
                cs = c * 512
                nc.scalar.copy(
                    out_qk[0:DK, 0 * n_seq + cs: 0 * n_seq + cs + 512], ps[0:DK, :])
                nc.scalar.copy(
                    out_qk[DK:P, 1 * n_seq + cs: 1 * n_seq + cs + 512], ps[DK:P, :])
                ps2 = vpsum.tile([P, 512], FP32, tag="pv")
                for it in range(DMT):
                    nc.tensor.matmul(
                        ps2[0:DK, :], lhsT=wb[:, it, 2 * DK:HD], rhs=xb[:, it, :],
                        start=(it == 0), stop=(it == DMT - 1))
                nc.scalar.copy(
                    out_qk[0:DK, 2 * n_seq + cs: 2 * n_seq + cs + 512], ps2[0:DK, :])
            if do_v:
                for st in range(4):
                    kt = c * 4 + st
                    ps = vpsum.tile([P, 512], FP32, tag="pv")
                    for it in range(DMT):
                        nc.tensor.matmul(
                            ps[:, 0:HD],
                            lhsT=xb[:, it, st * P:(st + 1) * P],
                            rhs=w_bf["v"][:, it, :],
                            start=(it == 0), stop=(it == DMT - 1))
                    nc.vector.tensor_copy(V[:, kt * HD:(kt + 1) * HD], ps[:, 0:HD])

    proj_chunks(xqT, Q, QT, False)
    proj_chunks(xkT, S, KT, False)
    proj_chunks(xvT, S, None, True)

    # mirror each head's projection to the other partition half
    # (head 1 was produced at partitions 64-127; heads 0,2 at 0-63)
    for buf, n_seq in ((QT, Q), (KT, S)):
        nc.sync.dma_start(buf[DK:P, 0:n_seq], buf[0:DK, 0:n_seq])
        nc.sync.dma_start(buf[0:DK, n_seq:2 * n_seq], buf[DK:P, n_seq:2 * n_seq])
        nc.sync.dma_start(buf[DK:P, 2 * n_seq:3 * n_seq], buf[0:DK, 2 * n_seq:3 * n_seq])

    # --- attention ---
    pending_wo = []
    for qc in range(4):                     # query chunks of 512
        outT_sb = []
        for h in range(HP):
            if h == 1 and pending_wo:
                pending_wo.pop(0)()
            anT = anT_p.tile([P, 4, NKT, P], BF16, tag="anT")
            for ql in range(4):             # 128-query tiles in this chunk
                qt = qc * 4 + ql
                un = unorm_p.tile([P, S], BF16, tag="unorm")
                dparts = dpool.tile([P, 4], FP32, tag="dparts")
                for kc in range(4):         # key chunks of 1024
                    sp = spsum.tile([P, 1024], FP32, tag="sc")
                    for half in range(2):
                        ks = h * S + kc * 1024 + half * 512
                        # diagonal pack: q rows 0-63 on PE quadrant (0,0),
                        # q rows 64-127 on (64,64) -> disjoint sub-arrays
                        # AND disjoint output partitions (concurrent-safe)
                        nc.tensor.matmul(
                            sp[0:DK, half * 512:(half + 1) * 512],
                            lhsT=QT[0:DK, h * Q + qt * P: h * Q + qt * P + DK],
                            rhs=KT[0:DK, ks:ks + 512],
                            start=True, stop=True,
                            tile_position=(0, 0))
                        nc.tensor.matmul(
                            sp[DK:P, half * 512:(half + 1) * 512],
                            lhsT=QT[DK:P,
                                    h * Q + qt * P + DK: h * Q + (qt + 1) * P],
                            rhs=KT[DK:P, ks:ks + 512],
                            start=True, stop=True,
                            tile_position=(DK, DK))
                    nc.scalar.activation(
                        un[:, kc * 1024:(kc + 1) * 1024], sp[:], Exp,
                        scale=0.125, accum_out=dparts[:, kc:kc + 1])
                den = dpool.tile([P, 1], FP32, tag="den")
                nc.vector.tensor_reduce(
                    den[:], dparts[:], axis=mybir.AxisListType.X,
                    op=mybir.AluOpType.add)
                rec = dpool.tile([P, 1], FP32, tag="rec")
                nc.vector.reciprocal(rec[:], den[:])
                an = anorm_p.tile([P, S], BF16, tag="anorm")
                nc.vector.tensor_scalar_mul(an[:], un[:], rec[:])
                nc.sync.dma_start(
                    attn_out[h * Q + qt * P: h * Q + (qt + 1) * P, :], an[:])
                if ql == 0 or (ql == 2 and h % 2 == 1):
                    # x-bar DMA transpose: [128q, 4096k] -> anT[:, 0, kt, q]
                    nc.sync.dma_start_transpose(anT[:, ql], an[:])
                else:
                    # PE transpose via regular matmul against identity
                    # (counts as PE activity -> keeps HAM clock warm)
                    for kt0 in range(0, NKT, 4):
                        tp = tpsum.tile([P, 512], FP32, tag="tp")
                        for j in range(4):
                            nc.tensor.matmul(
                                tp[:, j * P:(j + 1) * P],
                                lhsT=an[:, (kt0 + j) * P:(kt0 + j + 1) * P],
                                rhs=ident[:],
                                start=True, stop=True)
                        nc.vector.tensor_copy(
                            anT[:, ql, kt0:kt0 + 4, :], tp[:])

            # PV for this (head, query chunk)
            ov = vpsum.tile([P, 512], FP32, tag="pv")
            for kt in range(NKT):
                cb = DK * (kt % 2)
                nc.tensor.matmul(
                    ov[cb:cb + DK, :],
                    lhsT=V[:, kt * HD + h * DK: kt * HD + (h + 1) * DK],
                    rhs=anT[:, :, kt, :],
                    start=(kt < 2), stop=(kt >= NKT - 2),
                    tile_position=(0, cb),
                    skip_group_check=True)
            # merge even/odd k-tile chains: partition-shift via DMA, then add
            osb = outT_p.tile([P, 512], FP32, tag="outT")
            nc.vector.tensor_copy(osb[:], ov[:])
            oshift = outT_p.tile([DK, 512], FP32, tag="oshift")
            nc.sync.dma_start(oshift[:], osb[DK:P, :])
            omrg = omrg_p.tile([DK, 512], BF16, tag="omrg")
            nc.vector.tensor_add(omrg[:], osb[0:DK, :], oshift[:])
            outT_sb.append(omrg)

        # w_o projection, deferred (emitted after next chunk's first head
        # so the PE queue never idles at the chunk boundary)
        def make_wo(outT_sb_c, qc_c):
            def emit():
                for ql in range(4):
                    qt = qc_c * 4 + ql
                    of = ofin_p.tile([P, DM], FP32, tag="ofin")
                    for n0 in (0, 384):
                        wps = tpsum.tile([P, 512], FP32, tag="tp")
                        for h in range(HP):
                            nc.tensor.matmul(
                                wps[:, 0:384],
                                lhsT=outT_sb_c[h][:, ql * P:(ql + 1) * P],
                                rhs=wo_bf[h][:, n0:n0 + 384],
                                start=(h == 0), stop=(h == HP - 1),
                                tile_position=(0, 0))
                        nc.vector.tensor_copy(of[:, n0:n0 + 384], wps[:, 0:384])
                    nc.sync.dma_start(out_part[qt * P:(qt + 1) * P, :], of[:])
            return emit
        pending_wo.append(make_wo(outT_sb, qc))
    while pending_wo:
        pending_wo.pop(0)()


_NC_CACHE = None


def _get_nc():
    global _NC_CACHE
    if _NC_CACHE is None:
        _NC_CACHE = build_kernel()
    return _NC_CACHE


def _prep_inputs(query, key, value, w_q, w_k, w_v, w_o):
    import ml_dtypes

    bf16 = ml_dtypes.bfloat16
    q2 = np.asarray(query, np.float32).reshape(S, DM)
    k2 = np.asarray(key, np.float32).reshape(S, DM)
    v2 = np.asarray(value, np.float32).reshape(S, DM)
    w_q = np.asarray(w_q, np.float32)
    w_k = np.asarray(w_k, np.float32)
    w_v = np.asarray(w_v, np.float32)
    w_o = np.asarray(w_o, np.float32)

    xkT = np.ascontiguousarray(k2.T).astype(bf16).reshape(DMT, P, S)
    xvT = np.ascontiguousarray(v2.T).astype(bf16).reshape(DMT, P, S)
    xqT_half = [
        np.ascontiguousarray(q2[s * Q:(s + 1) * Q].T).astype(bf16).reshape(DMT, P, Q)
        for s in range(2)
    ]
    in_maps = []
    for core in range(8):
        g, s = divmod(core, 2)
        wq_g = np.ascontiguousarray(w_q[g * HD:(g + 1) * HD].T).astype(bf16).reshape(DMT, P, HD)
        wk_g = np.ascontiguousarray(w_k[g * HD:(g + 1) * HD].T).astype(bf16).reshape(DMT, P, HD)
        wv_g = np.ascontiguousarray(w_v[g * HD:(g + 1) * HD].T).astype(bf16).reshape(DMT, P, HD)
        wo_g = np.ascontiguousarray(w_o[:, g * HD:(g + 1) * HD].T).astype(bf16)
        in_maps.append({
            "xqT": xqT_half[s], "xkT": xkT, "xvT": xvT,
            "wqT": wq_g, "wkT": wk_g, "wvT": wv_g, "woT": wo_g,
        })
    return in_maps


def kernel(query, key, value, mask, w_q, w_k, w_v, w_o, _trace=False):
    # mask is all-False by construction (spec fill=zeros) -> no-op in softmax.
    nc = _get_nc()
    in_maps = _prep_inputs(query, key, value, w_q, w_k, w_v, w_o)
    res = run_bass_kernel_spmd(nc, in_maps, list(range(8)), trace=_trace)
    outs = res.results

    attn = np.empty((NH, S, S), np.float32)
    out = np.zeros((S, DM), np.float32)
    for core in range(8):
        g, s = divmod(core, 2)
        attn[g * HP:(g + 1) * HP, s * Q:(s + 1) * Q, :] = (
            outs[core]["attn_out"].reshape(HP, Q, S).astype(np.float32))
        out[s * Q:(s + 1) * Q] += outs[core]["out_part"]

    result = (out.reshape(1, S, DM), attn.reshape(1, NH, S, S))
    if _trace:
        return result, res
    return result


# revision 33
# speedup vs baseline: 1.1841x; 1.1841x over previous
"""Multi-head attention (B=1, S=4096, d_model=768, 12 heads) on 8 trn2 NeuronCores.

Sharding: tensor-parallel over heads (4 groups x 3 heads) x sequence-parallel over
queries (2 halves of 2048).  Core c = (g, s) with g = c // 2, s = c % 2 computes
heads 3g..3g+2 for queries s*2048..(s+1)*2048 against all 4096 keys.

Per-core device pipeline (all matmuls bf16, fp32 PSUM accumulation):
  1. Project Q^T, K^T, V from host-transposed bf16 activations; Q/K head pairs
     col-packed on disjoint PE quadrants (concurrent sub-array matmuls must
     write disjoint PSUM partitions - overlapping-output row-packing faults).
  2. scores = Q K^T per (head, 128-query tile), diagonally packed (quadrants
     (0,0)+(64,64)); exp on ScalarE with fused 1/8 scale and fused row-sum
     accumulation (softmax denominators); max-subtraction skipped (scores
     ~ N(0,1), exp is range-safe in fp32).
  3. Normalize with DVE tensor_scalar (4x bf16 mode) -> attn to HBM as bf16.
  4. Transpose attn for PV: 1/4 via DMA x-bar transpose, 3/4 via regular
     matmuls against identity (transpose_mode does not count as PE activity
     for the HAM clock gate and chills the clock to 1.2 GHz; real matmuls
     keep it at 2.4).  PV even/odd-ktile column packing -> out^T per head.
  5. w_o projection (K=64 chains), emission deferred past the next query
     chunk's first head so the PE queue never drains at chunk boundaries.

Host: slices/transposes/bf16-casts inputs (layout only), gathers attn shards
(bf16->fp32), sums the 4 head-group partials of out (the w_o "all-reduce").
"""

import numpy as np

import concourse.bass as bass
import concourse.tile as tile
from concourse import mybir
from concourse.bass_utils import run_bass_kernel_spmd
from concourse.masks import make_identity

FP32 = mybir.dt.float32
BF16 = mybir.dt.bfloat16

DM = 768          # d_model
NH = 12           # total heads
DK = 64           # head dim
HP = 3            # heads per core
HD = HP * DK      # 192: head dims per core
S = 4096          # keys (full sequence)
Q = 2048          # queries per core
P = 128
NQT = Q // P      # 16 query tiles
NKT = S // P      # 32 key tiles
DMT = DM // P     # 6 contraction tiles


def _split_multiwaits(nc):
    """The walrus in this container encodes at most one sync wait per
    instruction; split extra waits onto same-engine NoOps placed before."""
    for blk in nc.m.functions[0].blocks:
        new = []
        for inst in blk.instructions:
            si = inst.sync_info
            if si is not None and si.on_wait and len(si.on_wait) > 1:
                waits = list(si.on_wait)
                for extra in waits[:-1]:
                    nop = mybir.InstNoOp(name=f"WS-{nc.next_id()}", ins=[], outs=[])
                    nop.engine = inst.engine
                    nop.sync_info = mybir.SyncInfo(on_wait=[extra], on_update=[])
                    new.append(nop)
                inst.sync_info = mybir.SyncInfo(
                    on_wait=[waits[-1]], on_update=list(si.on_update))
            new.append(inst)
        blk.instructions = new


def build_kernel(split=True):
    nc = bass.Bass(trn_type="TRN2")

    xqT = nc.dram_tensor("xqT", [DMT, P, Q], BF16, kind="ExternalInput")
    xkT = nc.dram_tensor("xkT", [DMT, P, S], BF16, kind="ExternalInput")
    xvT = nc.dram_tensor("xvT", [DMT, P, S], BF16, kind="ExternalInput")
    wqT = nc.dram_tensor("wqT", [DMT, P, HD], BF16, kind="ExternalInput")
    wkT = nc.dram_tensor("wkT", [DMT, P, HD], BF16, kind="ExternalInput")
    wvT = nc.dram_tensor("wvT", [DMT, P, HD], BF16, kind="ExternalInput")
    woT = nc.dram_tensor("woT", [HD, DM], BF16, kind="ExternalInput")

    attn_out = nc.dram_tensor("attn_out", [HP * Q, S], BF16, kind="ExternalOutput")
    out_part = nc.dram_tensor("out_part", [Q, DM], FP32, kind="ExternalOutput")

    with tile.TileContext(nc) as tc:
        from contextlib import ExitStack

        with ExitStack() as ctx:
            kernel_body(ctx, tc, nc, xqT, xkT, xvT, wqT, wkT, wvT, woT,
                        attn_out, out_part)
    if split:
        _split_multiwaits(nc)
    return nc


def kernel_body(ctx, tc, nc, xqT, xkT, xvT, wqT, wkT, wvT, woT,
                attn_out, out_part):
    Exp = mybir.ActivationFunctionType.Exp

    wpool = ctx.enter_context(tc.tile_pool(name="wpool", bufs=1))
    xbf = ctx.enter_context(tc.tile_pool(name="xbf", bufs=3))
    qkv = ctx.enter_context(tc.tile_pool(name="qkv", bufs=1))
    unorm_p = ctx.enter_context(tc.tile_pool(name="unorm", bufs=2))
    anorm_p = ctx.enter_context(tc.tile_pool(name="anorm", bufs=3))
    anT_p = ctx.enter_context(tc.tile_pool(name="anT", bufs=2))
    outT_p = ctx.enter_context(tc.tile_pool(name="outT", bufs=2))
    omrg_p = ctx.enter_context(tc.tile_pool(name="omrg", bufs=4))
    ofin_p = ctx.enter_context(tc.tile_pool(name="ofin", bufs=2))
    dpool = ctx.enter_context(tc.tile_pool(name="dpool", bufs=8))

    spsum = ctx.enter_context(tc.tile_pool(name="spsum", bufs=2, space="PSUM"))
    tpsum = ctx.enter_context(tc.tile_pool(name="tpsum", bufs=2, space="PSUM"))
    vpsum = ctx.enter_context(tc.tile_pool(name="vpsum", bufs=2, space="PSUM"))

    const = ctx.enter_context(tc.tile_pool(name="const", bufs=1))
    ident = const.tile([P, P], BF16)
    make_identity(nc, ident)

    # --- load weights (host pre-cast to bf16) ---
    w_bf = {}
    for name, wsrc in (("q", wqT), ("k", wkT), ("v", wvT)):
        wb = wpool.tile([P, DMT, HD], BF16, tag=f"w{name}")
        nc.sync.dma_start(wb[:], wsrc.rearrange("t p c -> p t c"))
        w_bf[name] = wb

    wo_bf = []
    for h in range(HP):
        wob = wpool.tile([DK, DM], BF16, tag=f"wo{h}")
        nc.sync.dma_start(wob[:], woT[h * DK:(h + 1) * DK, :])
        wo_bf.append(wob)

    # --- resident activations ---
    QT = qkv.tile([P, HP * Q], BF16, tag="QT")    # [64 lo | 64 hi dup], per head
    KT = qkv.tile([P, HP * S], BF16, tag="KT")
    V = qkv.tile([P, NKT * HD], BF16, tag="V")    # per ktile: [128 k, 192 hd]

    # --- projections: stream x^T chunks of 512 seq positions ---
    def proj_chunks(src, n_seq, out_qk, do_v):
        n_ch = n_seq // 512
        for c in range(n_ch):
            xb = xbf.tile([P, DMT, 512], BF16, tag="xbf")
            nc.sync.dma_start(
                xb[:], src[:, :, c * 512:(c + 1) * 512].rearrange("t p c -> p t c"))
            if out_qk is not None:
                wb = w_bf["q" if n_seq == Q else "k"]
                # heads 0+1 col-packed (disjoint output partitions), head 2 solo
                ps = vpsum.tile([P, 512], FP32, tag="pv")
                for it in range(DMT):
                    nc.tensor.matmul(
                        ps[0:DK, :], lhsT=wb[:, it, 0:DK], rhs=xb[:, it, :],
                        start=(it == 0), stop=(it == DMT - 1),
                        tile_position=(0, 0), skip_group_check=True)
                    nc.tensor.matmul(
                        ps[DK:P, :], lhsT=wb[:, it, DK:2 * DK], rhs=xb[:, it, :],
                        start=(it == 0), stop=(it == DMT - 1),
                        tile_position=(0, DK), skip_group_check=True)
                cs = c * 512
                nc.scalar.copy(
                    out_qk[0:DK, 0 * n_seq + cs: 0 * n_seq + cs + 512], ps[0:DK, :])
                nc.scalar.copy(
                    out_qk[DK:P, 1 * n_seq + cs: 1 * n_seq + cs + 512], ps[DK:P, :])
                ps2 = vpsum.tile([P, 512], FP32, tag="pv")
                for it in range(DMT):
                    nc.tensor.matmul(
                        ps2[0:DK, :], lhsT=wb[:, it, 2 * DK:HD], rhs=xb[:, it, :],
                        start=(it == 0), stop=(it == DMT - 1))
                nc.scalar.copy(
                    out_qk[0:DK, 2 * n_seq + cs: 2 * n_seq + cs + 512], ps2[0:DK, :])
            if do_v:
                for st in range(4):
                    kt = c * 4 + st
                    ps = vpsum.tile([P, 512], FP32, tag="pv")
                    for it in range(DMT):
                        nc.tensor.matmul(
                            ps[:, 0:HD],
                            lhsT=xb[:, it, st * P:(st + 1) * P],
                            rhs=w_bf["v"][:, it, :],
                            start=(it == 0), stop=(it == DMT - 1))
                    nc.scalar.copy(V[:, kt * HD:(kt + 1) * HD], ps[:, 0:HD])

    proj_chunks(xqT, Q, QT, False)
    proj_chunks(xkT, S, KT, False)
    proj_chunks(xvT, S, None, True)

    # mirror each head's projection to the other partition half
    # (head 1 was produced at partitions 64-127; heads 0,2 at 0-63)
    for buf, n_seq in ((QT, Q), (KT, S)):
        nc.sync.dma_start(buf[DK:P, 0:n_seq], buf[0:DK, 0:n_seq])
        nc.sync.dma_start(buf[0:DK, n_seq:2 * n_seq], buf[DK:P, n_seq:2 * n_seq])
        nc.sync.dma_start(buf[DK:P, 2 * n_seq:3 * n_seq], buf[0:DK, 2 * n_seq:3 * n_seq])

    # --- attention ---
    pending_wo = []
    for qc in range(4):                     # query chunks of 512
        outT_sb = []
        for h in range(HP):
            if h == 1 and pending_wo:
                pending_wo.pop(0)()
            anT = anT_p.tile([P, 4, NKT, P], BF16, tag="anT")
            pend_tr = []
            for ql in range(4):             # 128-query tiles in this chunk
                qt = qc * 4 + ql
                if ql >= 2 and pend_tr:
                    pend_tr.pop(0)()
                un = unorm_p.tile([P, S], BF16, tag="unorm")
                dparts = dpool.tile([P, 4], FP32, tag="dparts")
                for kc in range(4):         # key chunks of 1024
                    sp = spsum.tile([P, 1024], FP32, tag="sc")
                    for half in range(2):
                        ks = h * S + kc * 1024 + half * 512
                        # diagonal pack: q rows 0-63 on PE quadrant (0,0),
                        # q rows 64-127 on (64,64) -> disjoint sub-arrays
                        # AND disjoint output partitions (concurrent-safe)
                        nc.tensor.matmul(
                            sp[0:DK, half * 512:(half + 1) * 512],
                            lhsT=QT[0:DK, h * Q + qt * P: h * Q + qt * P + DK],
                            rhs=KT[0:DK, ks:ks + 512],
                            start=True, stop=True,
                            tile_position=(0, 0))
                        nc.tensor.matmul(
                            sp[DK:P, half * 512:(half + 1) * 512],
                            lhsT=QT[DK:P,
                                    h * Q + qt * P + DK: h * Q + (qt + 1) * P],
                            rhs=KT[DK:P, ks:ks + 512],
                            start=True, stop=True,
                            tile_position=(DK, DK))
                    nc.scalar.activation(
                        un[:, kc * 1024:(kc + 1) * 1024], sp[:], Exp,
                        scale=0.125, accum_out=dparts[:, kc:kc + 1])
                den = dpool.tile([P, 1], FP32, tag="den")
                nc.vector.tensor_reduce(
                    den[:], dparts[:], axis=mybir.AxisListType.X,
                    op=mybir.AluOpType.add)
                rec = dpool.tile([P, 1], FP32, tag="rec")
                nc.vector.reciprocal(rec[:], den[:])
                an = anorm_p.tile([P, S], BF16, tag="anorm")
                nc.vector.tensor_scalar_mul(an[:], un[:], rec[:])
                nc.sync.dma_start(
                    attn_out[h * Q + qt * P: h * Q + (qt + 1) * P, :], an[:])
                if ql == 0:
                    # x-bar DMA transpose: [128q, 4096k] -> anT[:, 0, kt, q]
                    nc.sync.dma_start_transpose(anT[:, ql], an[:])
                else:
                    # defer PE transposes; emitted between the NEXT query
                    # tile's score matmuls so the in-order PE queue has work
                    # during exp waits (keeps HAM warm, fills stalls)
                    def make_tr(an_c, ql_c, qt_c):
                        def emit():
                            # PE transpose via regular matmul vs identity
                            # (counts as PE activity for the HAM clock)
                            for kt0 in range(0, NKT, 4):
                                tp = tpsum.tile([P, 512], FP32, tag="tp")
                                for j in range(4):
                                    nc.tensor.matmul(
                                        tp[:, j * P:(j + 1) * P],
                                        lhsT=an_c[:, (kt0 + j) * P:
                                                  (kt0 + j + 1) * P],
                                        rhs=ident[:],
                                        start=True, stop=True)
                                if (qt_c + kt0 // 4) % 4 == 3:
                                    nc.scalar.copy(
                                        anT[:, ql_c, kt0:kt0 + 4, :], tp[:])
                                else:
                                    nc.vector.tensor_copy(
                                        anT[:, ql_c, kt0:kt0 + 4, :], tp[:])
                        return emit
                    pend_tr.append(make_tr(an, ql, qt))

            while pend_tr:
                pend_tr.pop(0)()

            # PV for this (head, query chunk)
            ov = vpsum.tile([P, 512], FP32, tag="pv")
            for kt in range(NKT):
                cb = DK * (kt % 2)
                nc.tensor.matmul(
                    ov[cb:cb + DK, :],
                    lhsT=V[:, kt * HD + h * DK: kt * HD + (h + 1) * DK],
                    rhs=anT[:, :, kt, :],
                    start=(kt < 2), stop=(kt >= NKT - 2),
                    tile_position=(0, cb),
                    skip_group_check=True)
            # merge even/odd k-tile chains: partition-shift via DMA, then add
            osb = outT_p.tile([P, 512], FP32, tag="outT")
            nc.vector.tensor_copy(osb[:], ov[:])
            oshift = outT_p.tile([DK, 512], FP32, tag="oshift")
            nc.sync.dma_start(oshift[:], osb[DK:P, :])
            omrg = omrg_p.tile([DK, 512], BF16, tag="omrg")
            nc.vector.tensor_add(omrg[:], osb[0:DK, :], oshift[:])
            outT_sb.append(omrg)

        # w_o projection, deferred (emitted after next chunk's first head
        # so the PE queue never idles at the chunk boundary)
        def make_wo(outT_sb_c, qc_c):
            def emit():
                for ql in range(4):
                    qt = qc_c * 4 + ql
                    of = ofin_p.tile([P, DM], FP32, tag="ofin")
                    for n0 in (0, 384):
                        wps = tpsum.tile([P, 512], FP32, tag="tp")
                        for h in range(HP):
                            nc.tensor.matmul(
                                wps[:, 0:384],
                                lhsT=outT_sb_c[h][:, ql * P:(ql + 1) * P],
                                rhs=wo_bf[h][:, n0:n0 + 384],
                                start=(h == 0), stop=(h == HP - 1),
                                tile_position=(0, 0))
                        nc.vector.tensor_copy(of[:, n0:n0 + 384], wps[:, 0:384])
                    nc.sync.dma_start(out_part[qt * P:(qt + 1) * P, :], of[:])
            return emit
        pending_wo.append(make_wo(outT_sb, qc))
    while pending_wo:
        pending_wo.pop(0)()


_NC_CACHE = None


def _get_nc():
    global _NC_CACHE
    if _NC_CACHE is None:
        _NC_CACHE = build_kernel()
    return _NC_CACHE


def _prep_inputs(query, key, value, w_q, w_k, w_v, w_o):
    import ml_dtypes

    bf16 = ml_dtypes.bfloat16
    q2 = np.asarray(query, np.float32).reshape(S, DM)
    k2 = np.asarray(key, np.float32).reshape(S, DM)
    v2 = np.asarray(value, np.float32).reshape(S, DM)
    w_q = np.asarray(w_q, np.float32)
    w_k = np.asarray(w_k, np.float32)
    w_v = np.asarray(w_v, np.float32)
    w_o = np.asarray(w_o, np.float32)

    xkT = np.ascontiguousarray(k2.T).astype(bf16).reshape(DMT, P, S)
    xvT = np.ascontiguousarray(v2.T).astype(bf16).reshape(DMT, P, S)
    xqT_half = [
        np.ascontiguousarray(q2[s * Q:(s + 1) * Q].T).astype(bf16).reshape(DMT, P, Q)
        for s in range(2)
    ]
    in_maps = []
    for core in range(8):
        g, s = divmod(core, 2)
        wq_g = np.ascontiguousarray(w_q[g * HD:(g + 1) * HD].T).astype(bf16).reshape(DMT, P, HD)
        wk_g = np.ascontiguousarray(w_k[g * HD:(g + 1) * HD].T).astype(bf16).reshape(DMT, P, HD)
        wv_g = np.ascontiguousarray(w_v[g * HD:(g + 1) * HD].T).astype(bf16).reshape(DMT, P, HD)
        wo_g = np.ascontiguousarray(w_o[:, g * HD:(g + 1) * HD].T).astype(bf16)
        in_maps.append({
            "xqT": xqT_half[s], "xkT": xkT, "xvT": xvT,
            "wqT": wq_g, "wkT": wk_g, "wvT": wv_g, "woT": wo_g,
        })
    return in_maps


def kernel(query, key, value, mask, w_q, w_k, w_v, w_o, _trace=False):
    # mask is all-False by construction (spec fill=zeros) -> no-op in softmax.
    nc = _get_nc()
    in_maps = _prep_inputs(query, key, value, w_q, w_k, w_v, w_o)
    res = run_bass_kernel_spmd(nc, in_maps, list(range(8)), trace=_trace)
    outs = res.results

    attn = np.empty((NH, S, S), np.float32)
    out = np.zeros((S, DM), np.float32)
    for core in range(8):
        g, s = divmod(core, 2)
        attn[g * HP:(g + 1) * HP, s * Q:(s + 1) * Q, :] = (
            outs[core]["attn_out"].reshape(HP, Q, S).astype(np.float32))
        out[s * Q:(s + 1) * Q] += outs[core]["out_part"]

    result = (out.reshape(1, S, DM), attn.reshape(1, NH, S, S))
    if _trace:
        return result, res
    return result


# revision 34
# speedup vs baseline: 1.1884x; 1.0036x over previous
"""Multi-head attention (B=1, S=4096, d_model=768, 12 heads) on 8 trn2 NeuronCores.

Sharding: tensor-parallel over heads (4 groups x 3 heads) x sequence-parallel over
queries (2 halves of 2048).  Core c = (g, s) with g = c // 2, s = c % 2 computes
heads 3g..3g+2 for queries s*2048..(s+1)*2048 against all 4096 keys.

Per-core device pipeline (all matmuls bf16, fp32 PSUM accumulation):
  1. Project Q^T, K^T, V from host-transposed bf16 activations; Q/K head pairs
     col-packed on disjoint PE quadrants (concurrent sub-array matmuls must
     write disjoint PSUM partitions - overlapping-output row-packing faults).
  2. scores = Q K^T per (head, 128-query tile), diagonally packed (quadrants
     (0,0)+(64,64)); exp on ScalarE with fused 1/8 scale and fused row-sum
     accumulation (softmax denominators); max-subtraction skipped (scores
     ~ N(0,1), exp is range-safe in fp32).
  3. Normalize with DVE tensor_scalar (4x bf16 mode) -> attn to HBM as bf16.
  4. Transpose attn for PV: 1/4 via DMA x-bar transpose, 3/4 via regular
     matmuls against identity (transpose_mode does not count as PE activity
     for the HAM clock gate and chills the clock to 1.2 GHz; real matmuls
     keep it at 2.4).  PV even/odd-ktile column packing -> out^T per head.
  5. w_o projection (K=64 chains), emission deferred past the next query
     chunk's first head so the PE queue never drains at chunk boundaries.

Host: slices/transposes/bf16-casts inputs (layout only), gathers attn shards
(bf16->fp32), sums the 4 head-group partials of out (the w_o "all-reduce").
"""

import numpy as np

import concourse.bass as bass
import concourse.tile as tile
from concourse import mybir
from concourse.bass_utils import run_bass_kernel_spmd
from concourse.masks import make_identity

FP32 = mybir.dt.float32
BF16 = mybir.dt.bfloat16

DM = 768          # d_model
NH = 12           # total heads
DK = 64           # head dim
HP = 3            # heads per core
HD = HP * DK      # 192: head dims per core
S = 4096          # keys (full sequence)
Q = 2048          # queries per core
P = 128
NQT = Q // P      # 16 query tiles
NKT = S // P      # 32 key tiles
DMT = DM // P     # 6 contraction tiles


def _split_multiwaits(nc):
    """The walrus in this container encodes at most one sync wait per
    instruction; split extra waits onto same-engine NoOps placed before."""
    for blk in nc.m.functions[0].blocks:
        new = []
        for inst in blk.instructions:
            si = inst.sync_info
            if si is not None and si.on_wait and len(si.on_wait) > 1:
                waits = list(si.on_wait)
                for extra in waits[:-1]:
                    nop = mybir.InstNoOp(name=f"WS-{nc.next_id()}", ins=[], outs=[])
                    nop.engine = inst.engine
                    nop.sync_info = mybir.SyncInfo(on_wait=[extra], on_update=[])
                    new.append(nop)
                inst.sync_info = mybir.SyncInfo(
                    on_wait=[waits[-1]], on_update=list(si.on_update))
            new.append(inst)
        blk.instructions = new


def build_kernel(split=True):
    nc = bass.Bass(trn_type="TRN2")

    xqT = nc.dram_tensor("xqT", [DMT, P, Q], BF16, kind="ExternalInput")
    xkT = nc.dram_tensor("xkT", [DMT, P, S], BF16, kind="ExternalInput")
    xvT = nc.dram_tensor("xvT", [DMT, P, S], BF16, kind="ExternalInput")
    wqT = nc.dram_tensor("wqT", [DMT, P, HD], BF16, kind="ExternalInput")
    wkT = nc.dram_tensor("wkT", [DMT, P, HD], BF16, kind="ExternalInput")
    wvT = nc.dram_tensor("wvT", [DMT, P, HD], BF16, kind="ExternalInput")
    woT = nc.dram_tensor("woT", [HD, DM], BF16, kind="ExternalInput")

    attn_out = nc.dram_tensor("attn_out", [HP * Q, S], BF16, kind="ExternalOutput")
    out_part = nc.dram_tensor("out_part", [Q, DM], FP32, kind="ExternalOutput")

    with tile.TileContext(nc) as tc:
        from contextlib import ExitStack

        with ExitStack() as ctx:
            kernel_body(ctx, tc, nc, xqT, xkT, xvT, wqT, wkT, wvT, woT,
                        attn_out, out_part)
    if split:
        _split_multiwaits(nc)
    return nc


def kernel_body(ctx, tc, nc, xqT, xkT, xvT, wqT, wkT, wvT, woT,
                attn_out, out_part):
    Exp = mybir.ActivationFunctionType.Exp

    wpool = ctx.enter_context(tc.tile_pool(name="wpool", bufs=1))
    xbf = ctx.enter_context(tc.tile_pool(name="xbf", bufs=3))
    qkv = ctx.enter_context(tc.tile_pool(name="qkv", bufs=1))
    unorm_p = ctx.enter_context(tc.tile_pool(name="unorm", bufs=2))
    anorm_p = ctx.enter_context(tc.tile_pool(name="anorm", bufs=3))
    anT_p = ctx.enter_context(tc.tile_pool(name="anT", bufs=2))
    outT_p = ctx.enter_context(tc.tile_pool(name="outT", bufs=2))
    omrg_p = ctx.enter_context(tc.tile_pool(name="omrg", bufs=4))
    ofin_p = ctx.enter_context(tc.tile_pool(name="ofin", bufs=2))
    dpool = ctx.enter_context(tc.tile_pool(name="dpool", bufs=8))

    spsum = ctx.enter_context(tc.tile_pool(name="spsum", bufs=2, space="PSUM"))
    tpsum = ctx.enter_context(tc.tile_pool(name="tpsum", bufs=2, space="PSUM"))
    vpsum = ctx.enter_context(tc.tile_pool(name="vpsum", bufs=2, space="PSUM"))

    const = ctx.enter_context(tc.tile_pool(name="const", bufs=1))
    ident = const.tile([P, P], BF16)
    make_identity(nc, ident)

    # --- load weights (host pre-cast to bf16) ---
    w_bf = {}
    for name, wsrc in (("q", wqT), ("k", wkT), ("v", wvT)):
        wb = wpool.tile([P, DMT, HD], BF16, tag=f"w{name}")
        nc.sync.dma_start(wb[:], wsrc.rearrange("t p c -> p t c"))
        w_bf[name] = wb

    wo_bf = []
    for h in range(HP):
        wob = wpool.tile([DK, DM], BF16, tag=f"wo{h}")
        nc.sync.dma_start(wob[:], woT[h * DK:(h + 1) * DK, :])
        wo_bf.append(wob)

    # --- resident activations ---
    QT = qkv.tile([P, HP * Q], BF16, tag="QT")    # [64 lo | 64 hi dup], per head
    KT = qkv.tile([P, HP * S], BF16, tag="KT")
    V = qkv.tile([P, NKT * HD], BF16, tag="V")    # per ktile: [128 k, 192 hd]

    # --- projections: stream x^T chunks of 512 seq positions ---
    def proj_chunks(src, n_seq, out_qk, do_v):
        n_ch = n_seq // 512
        for c in range(n_ch):
            xb = xbf.tile([P, DMT, 512], BF16, tag="xbf")
            nc.sync.dma_start(
                xb[:], src[:, :, c * 512:(c + 1) * 512].rearrange("t p c -> p t c"))
            if out_qk is not None:
                wb = w_bf["q" if n_seq == Q else "k"]
                # heads 0+1 col-packed (disjoint output partitions), head 2 solo
                ps = vpsum.tile([P, 512], FP32, tag="pv")
                for it in range(DMT):
                    nc.tensor.matmul(
                        ps[0:DK, :], lhsT=wb[:, it, 0:DK], rhs=xb[:, it, :],
                        start=(it == 0), stop=(it == DMT - 1),
                        tile_position=(0, 0), skip_group_check=True)
                    nc.tensor.matmul(
                        ps[DK:P, :], lhsT=wb[:, it, DK:2 * DK], rhs=xb[:, it, :],
                        start=(it == 0), stop=(it == DMT - 1),
                        tile_position=(0, DK), skip_group_check=True)
                cs = c * 512
                nc.scalar.copy(
                    out_qk[0:DK, 0 * n_seq + cs: 0 * n_seq + cs + 512], ps[0:DK, :])
                nc.scalar.copy(
                    out_qk[DK:P, 1 * n_seq + cs: 1 * n_seq + cs + 512], ps[DK:P, :])
                ps2 = vpsum.tile([P, 512], FP32, tag="pv")
                for it in range(DMT):
                    nc.tensor.matmul(
                        ps2[0:DK, :], lhsT=wb[:, it, 2 * DK:HD], rhs=xb[:, it, :],
                        start=(it == 0), stop=(it == DMT - 1))
                nc.scalar.copy(
                    out_qk[0:DK, 2 * n_seq + cs: 2 * n_seq + cs + 512], ps2[0:DK, :])
            if do_v:
                for st in range(4):
                    kt = c * 4 + st
                    ps = vpsum.tile([P, 512], FP32, tag="pv")
                    for it in range(DMT):
                        nc.tensor.matmul(
                            ps[:, 0:HD],
                            lhsT=xb[:, it, st * P:(st + 1) * P],
                            rhs=w_bf["v"][:, it, :],
                            start=(it == 0), stop=(it == DMT - 1))
                    nc.scalar.copy(V[:, kt * HD:(kt + 1) * HD], ps[:, 0:HD])

    proj_chunks(xqT, Q, QT, False)
    proj_chunks(xkT, S, KT, False)
    proj_chunks(xvT, S, None, True)

    # mirror each head's projection to the other partition half
    # (head 1 was produced at partitions 64-127; heads 0,2 at 0-63)
    for buf, n_seq in ((QT, Q), (KT, S)):
        nc.sync.dma_start(buf[DK:P, 0:n_seq], buf[0:DK, 0:n_seq])
        nc.sync.dma_start(buf[0:DK, n_seq:2 * n_seq], buf[DK:P, n_seq:2 * n_seq])
        nc.sync.dma_start(buf[DK:P, 2 * n_seq:3 * n_seq], buf[0:DK, 2 * n_seq:3 * n_seq])

    # --- attention ---
    pending_wo = []
    pend_pv = []
    for qc in range(4):                     # query chunks of 512
        outT_sb = []                        # filled by deferred PV closures
        for h in range(HP):
            if pend_pv:
                pend_pv.pop(0)()
            if h == 1 and pending_wo:
                pending_wo.pop(0)()
            anT = anT_p.tile([P, 4, NKT, P], BF16, tag="anT")
            pend_tr = []
            for ql in range(4):             # 128-query tiles in this chunk
                qt = qc * 4 + ql
                if ql >= 2 and pend_tr:
                    pend_tr.pop(0)()
                un = unorm_p.tile([P, S], BF16, tag="unorm")
                dparts = dpool.tile([P, 4], FP32, tag="dparts")
                for kc in range(4):         # key chunks of 1024
                    sp = spsum.tile([P, 1024], FP32, tag="sc")
                    for half in range(2):
                        ks = h * S + kc * 1024 + half * 512
                        # diagonal pack: q rows 0-63 on PE quadrant (0,0),
                        # q rows 64-127 on (64,64) -> disjoint sub-arrays
                        # AND disjoint output partitions (concurrent-safe)
                        nc.tensor.matmul(
                            sp[0:DK, half * 512:(half + 1) * 512],
                            lhsT=QT[0:DK, h * Q + qt * P: h * Q + qt * P + DK],
                            rhs=KT[0:DK, ks:ks + 512],
                            start=True, stop=True,
                            tile_position=(0, 0))
                        nc.tensor.matmul(
                            sp[DK:P, half * 512:(half + 1) * 512],
                            lhsT=QT[DK:P,
                                    h * Q + qt * P + DK: h * Q + (qt + 1) * P],
                            rhs=KT[DK:P, ks:ks + 512],
                            start=True, stop=True,
                            tile_position=(DK, DK))
                    nc.scalar.activation(
                        un[:, kc * 1024:(kc + 1) * 1024], sp[:], Exp,
                        scale=0.125, accum_out=dparts[:, kc:kc + 1])
                den = dpool.tile([P, 1], FP32, tag="den")
                nc.vector.tensor_reduce(
                    den[:], dparts[:], axis=mybir.AxisListType.X,
                    op=mybir.AluOpType.add)
                rec = dpool.tile([P, 1], FP32, tag="rec")
                nc.vector.reciprocal(rec[:], den[:])
                an = anorm_p.tile([P, S], BF16, tag="anorm")
                nc.vector.tensor_scalar_mul(an[:], un[:], rec[:])
                nc.sync.dma_start(
                    attn_out[h * Q + qt * P: h * Q + (qt + 1) * P, :], an[:])
                if ql == 0:
                    # x-bar DMA transpose: [128q, 4096k] -> anT[:, 0, kt, q]
                    nc.sync.dma_start_transpose(anT[:, ql], an[:])
                else:
                    # defer PE transposes; emitted between the NEXT query
                    # tile's score matmuls so the in-order PE queue has work
                    # during exp waits (keeps HAM warm, fills stalls)
                    def make_tr(an_c, ql_c, qt_c):
                        def emit():
                            # PE transpose via regular matmul vs identity
                            # (counts as PE activity for the HAM clock)
                            for kt0 in range(0, NKT, 4):
                                tp = tpsum.tile([P, 512], FP32, tag="tp")
                                for j in range(4):
                                    nc.tensor.matmul(
                                        tp[:, j * P:(j + 1) * P],
                                        lhsT=an_c[:, (kt0 + j) * P:
                                                  (kt0 + j + 1) * P],
                                        rhs=ident[:],
                                        start=True, stop=True)
                                if (qt_c + kt0 // 4) % 4 == 3:
                                    nc.scalar.copy(
                                        anT[:, ql_c, kt0:kt0 + 4, :], tp[:])
                                else:
                                    nc.vector.tensor_copy(
                                        anT[:, ql_c, kt0:kt0 + 4, :], tp[:])
                        return emit
                    pend_tr.append(make_tr(an, ql, qt))

            while pend_tr:
                pend_tr.pop(0)()

            # PV deferred one head: emitted after the next head's first
            # query tile so the in-order PE queue is never gated on the
            # attn-transpose copies
            def make_pv(anT_c, h_c, sink):
                def emit():
                    ov = vpsum.tile([P, 512], FP32, tag="pv")
                    for kt in range(NKT):
                        cb = DK * (kt % 2)
                        nc.tensor.matmul(
                            ov[cb:cb + DK, :],
                            lhsT=V[:, kt * HD + h_c * DK:
                                   kt * HD + (h_c + 1) * DK],
                            rhs=anT_c[:, :, kt, :],
                            start=(kt < 2), stop=(kt >= NKT - 2),
                            tile_position=(0, cb),
                            skip_group_check=True)
                    # merge even/odd chains: partition-shift DMA, then add
                    osb = outT_p.tile([P, 512], FP32, tag="outT")
                    nc.vector.tensor_copy(osb[:], ov[:])
                    oshift = outT_p.tile([DK, 512], FP32, tag="oshift")
                    nc.sync.dma_start(oshift[:], osb[DK:P, :])
                    omrg = omrg_p.tile([DK, 512], BF16, tag="omrg")
                    nc.vector.tensor_add(omrg[:], osb[0:DK, :], oshift[:])
                    sink.append(omrg)
                return emit
            pend_pv.append(make_pv(anT, h, outT_sb))

        # w_o projection, deferred (emitted after next chunk's first head
        # so the PE queue never idles at the chunk boundary)
        def make_wo(outT_sb_c, qc_c):
            def emit():
                for ql in range(4):
                    qt = qc_c * 4 + ql
                    of = ofin_p.tile([P, DM], FP32, tag="ofin")
                    for n0 in (0, 384):
                        wps = tpsum.tile([P, 512], FP32, tag="tp")
                        for h in range(HP):
                            nc.tensor.matmul(
                                wps[:, 0:384],
                                lhsT=outT_sb_c[h][:, ql * P:(ql + 1) * P],
                                rhs=wo_bf[h][:, n0:n0 + 384],
                                start=(h == 0), stop=(h == HP - 1),
                                tile_position=(0, 0))
                        nc.vector.tensor_copy(of[:, n0:n0 + 384], wps[:, 0:384])
                    nc.sync.dma_start(out_part[qt * P:(qt + 1) * P, :], of[:])
            return emit
        pending_wo.append(make_wo(outT_sb, qc))
    while pend_pv:
        pend_pv.pop(0)()
    while pending_wo:
        pending_wo.pop(0)()


_NC_CACHE = None


def _get_nc():
    global _NC_CACHE
    if _NC_CACHE is None:
        _NC_CACHE = build_kernel()
    return _NC_CACHE


def _prep_inputs(query, key, value, w_q, w_k, w_v, w_o):
    import ml_dtypes

    bf16 = ml_dtypes.bfloat16
    q2 = np.asarray(query, np.float32).reshape(S, DM)
    k2 = np.asarray(key, np.float32).reshape(S, DM)
    v2 = np.asarray(value, np.float32).reshape(S, DM)
    w_q = np.asarray(w_q, np.float32)
    w_k = np.asarray(w_k, np.float32)
    w_v = np.asarray(w_v, np.float32)
    w_o = np.asarray(w_o, np.float32)

    xkT = np.ascontiguousarray(k2.T).astype(bf16).reshape(DMT, P, S)
    xvT = np.ascontiguousarray(v2.T).astype(bf16).reshape(DMT, P, S)
    xqT_half = [
        np.ascontiguousarray(q2[s * Q:(s + 1) * Q].T).astype(bf16).reshape(DMT, P, Q)
        for s in range(2)
    ]
    in_maps = []
    for core in range(8):
        g, s = divmod(core, 2)
        wq_g = np.ascontiguousarray(w_q[g * HD:(g + 1) * HD].T).astype(bf16).reshape(DMT, P, HD)
        wk_g = np.ascontiguousarray(w_k[g * HD:(g + 1) * HD].T).astype(bf16).reshape(DMT, P, HD)
        wv_g = np.ascontiguousarray(w_v[g * HD:(g + 1) * HD].T).astype(bf16).reshape(DMT, P, HD)
        wo_g = np.ascontiguousarray(w_o[:, g * HD:(g + 1) * HD].T).astype(bf16)
        in_maps.append({
            "xqT": xqT_half[s], "xkT": xkT, "xvT": xvT,
            "wqT": wq_g, "wkT": wk_g, "wvT": wv_g, "woT": wo_g,
        })
    return in_maps


def kernel(query, key, value, mask, w_q, w_k, w_v, w_o, _trace=False):
    # mask is all-False by construction (spec fill=zeros) -> no-op in softmax.
    nc = _get_nc()
    in_maps = _prep_inputs(query, key, value, w_q, w_k, w_v, w_o)
    res = run_bass_kernel_spmd(nc, in_maps, list(range(8)), trace=_trace)
    outs = res.results

    attn = np.empty((NH, S, S), np.float32)
    out = np.zeros((S, DM), np.float32)
    for core in range(8):
        g, s = divmod(core, 2)
        attn[g * HP:(g + 1) * HP, s * Q:(s + 1) * Q, :] = (
            outs[core]["attn_out"].reshape(HP, Q, S).astype(np.float32))
        out[s * Q:(s + 1) * Q] += outs[core]["out_part"]

    result = (out.reshape(1, S, DM), attn.reshape(1, NH, S, S))
    if _trace:
        return result, res
    return result
